# revision 73
# baseline (speedup 1.0000x reference)
"""Bass/Trainium2 kernel for nn_NaryTreeLSTM (binary TreeLSTM over a complete
depth-16 tree, H=D=256, heap/level node order).

Sharding: data-parallel over 8 independent subtrees. Core m owns the subtree
rooted at level-3 node m; within every level l the core's nodes are a
contiguous position block whose children stay in the core's block at level
l+1 — zero inter-core communication. The device computes levels 15..CUT per
core in F-layout; the top of the tree (2^CUT-1 nodes, ~3% of FLOPs) is
finished on host during the gather/unshard step (those levels are
latency-bound serial remnants that cost more in device sync than they are
worth).

Layout (per level, nodes stored in bit-reversed position order so the
even/odd children of a contiguous parent chunk are the first/second half of
the child level — no strided gathers): feature-on-partition, nodes-on-free.
W-tiles stationary in bf16 (full-rate PE, half-cost LDWEIGHTS), node columns
moving (N<=512/matmul). ACT applies sigmoid/tanh with the per-feature bias;
DVE does the c/h elementwise work. c stays fp32; h and gates are bf16.

Per node (children h_e,h_o / c_e,c_o; x = emb row):
  i = sig(Wi x + bi + Ui0 h_e + Ui1 h_o)      o, u analogous (u: tanh)
  f0 = sig(Wf x + bf + Uf0 h_e),  f1 = sig(Wf x + bf + Uf1 h_o)
  c = i*u + f0*c_e + f1*c_o ;  h = o * tanh(c)
"""

import os

import ml_dtypes
import numpy as np

NP_BF16 = ml_dtypes.bfloat16

try:
    import concourse  # noqa: F401
except ImportError:  # pragma: no cover
    import sys

    sys.path.insert(0, "/opt/trn_rl_repo")

import concourse.tile as tile
from concourse import bacc, mybir
from concourse.bass_utils import run_bass_kernel_spmd

F32 = mybir.dt.float32
BF16 = mybir.dt.bfloat16
FP8 = mybir.dt.float8e4
DR = mybir.MatmulPerfMode.DoubleRow
AF = mybir.ActivationFunctionType
NP_FP8 = ml_dtypes.float8_e4m3

DEPTH = 16
H = 256
P = 128
NCORES = 8
LTOP = DEPTH - 1
CUT = 12  # device computes levels 15..CUT; host finishes 2^CUT-1 top nodes

N_L = {l: 1 << (l - 3) for l in range(CUT, LTOP + 1)}
NSLOT = sum(N_L.values())
OFF = {}
_o = 0
for _l in range(LTOP, CUT - 1, -1):
    OFF[_l] = _o
    _o += N_L[_l]
NOUT = N_L[CUT]

# F-layout weight tables: wt8 = fp8 weights [Wi, Wo, Wu, Wf, Uo0, Uo1]
# (DoubleRow perf mode: both ko-halves of the contraction in one matmul).
# The o-gate's U terms run fp8 off an fp8 shadow of h: o only scales
# tanh(c) once (no recurrence), so its quantization error stays bounded.
# wtb = bf16 child-h weights [Ui0, Ui1, Uo0, Uo1, Uu0, Uu1, Uf0, Uf1]
W_I8, W_O8, W_U8, W_F8, U_O0_8, U_O1_8 = range(6)
U_I0_8, U_I1_8, U_U0_8, U_U1_8, U_F0_8, U_F1_8 = range(6, 12)
U_I0, U_I1, U_O0, U_O1, U_U0, U_U1, U_F0, U_F1 = range(8)
CHUNK = 512
NBLK = (NSLOT + 511) // 512


def _chunk_order(nch):
    """Process first/second-half chunks alternately so the parent level's
    chunk j (which needs child cols [j] and [half+j]) unblocks after two
    child chunks instead of half the level."""
    h = nch // 2
    return [i // 2 if i % 2 == 0 else h + i // 2 for i in range(nch)] if h else [0]


def _bitrev(nbits):
    n = 1 << nbits
    r = np.zeros(n, dtype=np.int64)
    for j in range(n):
        v = 0
        for b in range(nbits):
            if j & (1 << b):
                v |= 1 << (nbits - 1 - b)
        r[j] = v
    return r


def _build_program():
    nc = bacc.Bacc("TRN2", target_bir_lowering=False, debug=False, num_devices=NCORES)
    xtb = nc.dram_tensor("xtb", [P, 2, NBLK * 512], FP8, kind="ExternalInput").ap()
    wt8 = nc.dram_tensor("wt8", [P, 12, 2, 2, P], FP8, kind="ExternalInput").ap()
    wtb = nc.dram_tensor("wtb", [P, 8, 2, 2, P], BF16, kind="ExternalInput").ap()
    bs = nc.dram_tensor("bs", [P, 4, 2], F32, kind="ExternalInput").ap()
    praw = nc.dram_tensor("praw", [P, 5, 2, NOUT], BF16, kind="ExternalOutput").ap()
    hcc = nc.dram_tensor("hcc", [P, 2, 2 * NOUT], F32, kind="ExternalOutput").ap()

    with tile.TileContext(nc) as tc:
        with (
            tc.tile_pool(name="const", bufs=1) as const,
            tc.tile_pool(name="xp", bufs=2) as xp,
            tc.tile_pool(name="fstate", bufs=1) as fstate,
            tc.tile_pool(name="ps6", bufs=8, space="PSUM") as ps6,
        ):
            wt8_sb = const.tile([P, 12, 2, 2, P], FP8)
            bs_sb = const.tile([P, 4, 2], F32)

            def f_level(lvl, h_prev, c_prev, h8_prev, wtb_sb, fgp, fgp1, on_chunk=None):
                n = N_L[lvl]
                h_cur = fstate.tile([P, 2, n], BF16, tag=f"h{lvl % 2}", name="h")
                c_cur = fstate.tile([P, 2, n], F32, tag=f"c{lvl % 2}", name="c")
                h8_cur = (
                    fstate.tile([P, 2, n], FP8, tag=f"h8{lvl % 2}", name="h8")
                    if lvl > CUT else None
                )
                nch = (n + CHUNK - 1) // CHUNK
                for ci, cidx in enumerate(_chunk_order(nch)):
                    s = cidx * CHUNK
                    ch = min(CHUNK, n - s)
                    e = s + ch
                    if on_chunk is not None:
                        on_chunk(ci, c_cur, c_prev)
                    xt_t = xp.tile([P, 2, CHUNK], FP8, tag="x", name="x")
                    x0 = OFF[lvl] + s
                    nc.sync.dma_start(xt_t[:, :, :ch], xtb[:, :, x0 : x0 + ch])
                    # Gate matmuls+ACTs for BOTH mo halves are emitted before
                    # either half's tanh(c)/h ops: the in-order ACT queue then
                    # never parks on a tanh waiting for the DVE c-chain while
                    # ready gate ACTs sit behind it. The parent level consumes
                    # both mo halves of a column together, so deferring h
                    # costs no downstream latency.
                    deferred = []
                    for mo in range(2):

                        def gate(xg, usrcs, g_idx, func, tag):
                            pt = ps6.tile([P, CHUNK], F32, tag="ps", name="ps")[:, :ch]
                            nus = len(usrcs) * 2
                            # x-projection: both ko halves in one fp8 matmul
                            nc.tensor.matmul(
                                pt,
                                lhsT=wt8_sb[:, xg, :, mo],
                                rhs=xt_t[:, :, :ch],
                                start=True,
                                stop=(nus == 0),
                                perf_mode=DR,
                            )
                            k = 0
                            for wsb, w_idx, rhs in usrcs:
                                for ko in range(2):
                                    nc.tensor.matmul(
                                        pt,
                                        lhsT=wsb[:, w_idx, ko, mo],
                                        rhs=rhs(ko),
                                        start=False,
                                        stop=(k == nus - 1),
                                    )
                                    k += 1
                            sb = fgp.tile([P, CHUNK], BF16, tag=tag, name=tag)[:, :ch]
                            nc.scalar.activation(
                                sb, pt, func, bias=bs_sb[:, g_idx, mo : mo + 1]
                            )
                            return sb

                        if lvl == LTOP:
                            i_sb = gate(W_I8, [], 0, AF.Sigmoid, "gi")
                            u_sb = gate(W_U8, [], 2, AF.Tanh, "gu")
                            o_sb = gate(W_O8, [], 1, AF.Sigmoid, "go")
                            c_ap = c_cur[:, mo, s:e]
                            nc.vector.tensor_mul(out=c_ap, in0=i_sb, in1=u_sb)
                        else:
                            half = N_L[lvl + 1] // 2

                            def he(ko):
                                return h_prev[:, ko, s:e]

                            def ho(ko):
                                return h_prev[:, ko, half + s : half + e]

                            if lvl == CUT:
                                # final level: every gate is all-DR fp8 off
                                # the h8 shadow and ships its RAW preact
                                # (DVE psum-evac, no ACT); the host finishes
                                # c13/h13 elementwise
                                h8e = h8_prev[:, :, s:e]
                                h8o = h8_prev[:, :, half + s : half + e]

                                def raw_gate(terms, slot, tag):
                                    pt = ps6.tile([P, CHUNK], F32, tag="ps", name="ps")[:, :ch]
                                    for j, (w8, rhs8) in enumerate(terms):
                                        nc.tensor.matmul(
                                            pt, lhsT=wt8_sb[:, w8, :, mo],
                                            rhs=rhs8, start=(j == 0),
                                            stop=(j == len(terms) - 1),
                                            perf_mode=DR,
                                        )
                                    rb = fgp.tile([P, CHUNK], BF16, tag=tag, name=tag)[:, :ch]
                                    nc.vector.tensor_copy(rb, pt)
                                    nc.sync.dma_start(praw[:, slot, mo, s:e], rb)

                                xr = xt_t[:, :, :ch]
                                raw_gate([(W_I8, xr), (U_I0_8, h8e), (U_I1_8, h8o)], 0, "gi")
                                raw_gate([(W_U8, xr), (U_U0_8, h8e), (U_U1_8, h8o)], 1, "gu")
                                raw_gate([(W_F8, xr), (U_F0_8, h8e)], 2, "f0")
                                raw_gate([(W_F8, xr), (U_F1_8, h8o)], 3, "f1")
                                raw_gate([(W_O8, xr), (U_O0_8, h8e), (U_O1_8, h8o)], 4, "go")
                                continue

                            i_sb = gate(
                                W_I8, [(wtb_sb, U_I0, he), (wtb_sb, U_I1, ho)],
                                0, AF.Sigmoid, "gi",
                            )
                            u_sb = gate(
                                W_U8, [(wtb_sb, U_U0, he), (wtb_sb, U_U1, ho)],
                                2, AF.Tanh, "gu",
                            )
                            f0_sb = gate(
                                W_F8, [(wtb_sb, U_F0, he)],
                                3, AF.Sigmoid, "f0",
                            )
                            f1_sb = gate(
                                W_F8, [(wtb_sb, U_F1, ho)],
                                3, AF.Sigmoid, "f1",
                            )
                            # o last: its ACT hides the DVE c-chain tail so
                            # the tanh(c) that follows never stalls the queue.
                            # o's U terms are fp8 DoubleRow off the h8 shadow.
                            pt_o = ps6.tile([P, CHUNK], F32, tag="ps", name="ps")[:, :ch]
                            for j, (w8, rhs8) in enumerate((
                                (W_O8, xt_t[:, :, :ch]),
                                (U_O0_8, h8_prev[:, :, s:e]),
                                (U_O1_8, h8_prev[:, :, half + s : half + e]),
                            )):
                                nc.tensor.matmul(
                                    pt_o,
                                    lhsT=wt8_sb[:, w8, :, mo],
                                    rhs=rhs8,
                                    start=(j == 0),
                                    stop=(j == 2),
                                    perf_mode=DR,
                                )
                            o_sb = fgp.tile([P, CHUNK], BF16, tag="go", name="go")[:, :ch]
                            nc.scalar.activation(
                                o_sb, pt_o, AF.Sigmoid,
                                bias=bs_sb[:, 1, mo : mo + 1],
                            )
                            ce = c_prev[:, mo, s:e]
                            co = c_prev[:, mo, half + s : half + e]
                            iu = fgp1.tile([P, CHUNK], F32, tag="iu", name="iu")[:, :ch]
                            nc.vector.tensor_mul(out=iu, in0=i_sb, in1=u_sb)
                            t0 = fgp1.tile([P, CHUNK], F32, tag="t0", name="t0")[:, :ch]
                            nc.vector.tensor_mul(out=t0, in0=f0_sb, in1=ce)
                            t1 = fgp1.tile([P, CHUNK], F32, tag="t1", name="t1")[:, :ch]
                            nc.vector.tensor_mul(out=t1, in0=f1_sb, in1=co)
                            c_ap = c_cur[:, mo, s:e]
                            nc.vector.tensor_add(out=c_ap, in0=iu, in1=t0)
                            nc.vector.tensor_add(out=c_ap, in0=c_ap, in1=t1)

                        deferred.append((mo, c_ap, o_sb))

                    for mo, c_ap, o_sb in deferred:
                        if lvl == CUT + 1:
                            # stream l14's c out — the host needs it to
                            # finish the final level's cell state
                            nc.sync.dma_start(hcc[:, mo, s:e], c_ap)
                        th = fgp1.tile([P, CHUNK], BF16, tag="th", name="th")[:, :ch]
                        nc.scalar.activation(th, c_ap, AF.Tanh)
                        nc.vector.tensor_mul(out=h_cur[:, mo, s:e], in0=o_sb, in1=th)
                        if h8_cur is not None:
                            nc.vector.tensor_copy(
                                h8_cur[:, mo, s:e], h_cur[:, mo, s:e]
                            )
                return h_cur, c_cur, h8_cur

            # ---------------- tree walk ----------------
            with (
                tc.tile_pool(name="fwb", bufs=1) as fwb,
                tc.tile_pool(name="fgp", bufs=3) as fgp,
                tc.tile_pool(name="fgp1", bufs=3) as fgp1,
            ):
                wtb_sb = fwb.tile([P, 8, 2, 2, P], BF16)

                def on_chunk_leaf(ci, c_cur, c_prev):
                    # wt8/bs are emitted ahead of the first xt chunk so the
                    # first matmul's stationary weights arrive ASAP. The big
                    # U-table DMA is gated (WAW via a 1-elem copy that reads
                    # leaf c) so its HBM traffic cannot starve the leaf xt
                    # stream it would race with.
                    if ci == 0:
                        nc.sync.dma_start(wt8_sb[:], wt8)
                        nc.sync.dma_start(bs_sb[:], bs)
                    elif ci == 1:
                        nc.vector.tensor_copy(
                            wtb_sb[0:1, 0, 0, 0, 0:1], c_cur[0:1, 0, 0:1]
                        )
                        nc.sync.dma_start(wtb_sb[:], wtb)

                h_prev = c_prev = h8_prev = None
                for lvl in range(LTOP, CUT - 1, -1):
                    cb = on_chunk_leaf if lvl == LTOP else None
                    h_prev, c_prev, h8_prev = f_level(
                        lvl, h_prev, c_prev, h8_prev, wtb_sb, fgp, fgp1, on_chunk=cb
                    )

    nc.compile()
    return nc


_CACHE = {}


def _get_program():
    if "nc" not in _CACHE:
        _CACHE["nc"] = _build_program()
    return _CACHE["nc"]


def _core_index_table():
    if "idx" in _CACHE:
        return _CACHE["idx"]
    idx = np.zeros((NCORES, NSLOT), dtype=np.int64)
    for lvl in range(LTOP, CUT - 1, -1):
        n = N_L[lvl]
        rev = _bitrev(lvl - 3)
        start = (1 << lvl) - 1
        for m in range(NCORES):
            pos = m * n + rev
            idx[m, OFF[lvl] : OFF[lvl] + n] = start + pos
    _CACHE["idx"] = idx
    return idx


def _pack_w(mat):
    """[out,in] (256,256) -> [p, ko, mo, m] = W.T[ko*128+p, mo*128+m]."""
    return mat.reshape(2, P, 2, P).transpose(3, 2, 0, 1)


def _sigmoid(x):
    return 1.0 / (1.0 + np.exp(-x))


def _host_node_batch(x, ch_h, ch_c, prm):
    (Wi, bi, Ui, Wo, bo, Uo, Wu, bu, Uu, Wf, bf, Uf) = prm

    def gate(W, b, U):
        return x @ W.T + b + ch_h[:, 0] @ U[0].T + ch_h[:, 1] @ U[1].T

    i = _sigmoid(gate(Wi, bi, Ui))
    o = _sigmoid(gate(Wo, bo, Uo))
    u = np.tanh(gate(Wu, bu, Uu))
    xf = x @ Wf.T + bf
    f0 = _sigmoid(xf + ch_h[:, 0] @ Uf[0].T)
    f1 = _sigmoid(xf + ch_h[:, 1] @ Uf[1].T)
    c = i * u + f0 * ch_c[:, 0] + f1 * ch_c[:, 1]
    h = o * np.tanh(c)
    return h.astype(np.float32), c.astype(np.float32)


def kernel(emb, W_i, b_i, U_i, W_o, b_o, U_o, W_u, b_u, U_u, W_f, b_f, U_f):
    emb = np.asarray(emb, dtype=np.float32)
    f = lambda a: np.asarray(a, dtype=np.float32)
    W_i, b_i, U_i = f(W_i), f(b_i), f(U_i)
    W_o, b_o, U_o = f(W_o), f(b_o), f(U_o)
    W_u, b_u, U_u = f(W_u), f(b_u), f(U_u)
    W_f, b_f, U_f = f(W_f), f(b_f), f(U_f)

    nc = _get_program()
    idx = _core_index_table()

    wt8 = np.ascontiguousarray(
        np.stack(
            [
                _pack_w(m)
                for m in (
                    W_i, W_o, W_u, W_f, U_o[0], U_o[1],
                    U_i[0], U_i[1], U_u[0], U_u[1], U_f[0], U_f[1],
                )
            ],
            axis=1,
        )
    ).astype(NP_FP8)
    wtb = np.ascontiguousarray(
        np.stack(
            [
                _pack_w(m)
                for m in (
                    U_i[0], U_i[1], U_o[0], U_o[1], U_u[0], U_u[1],
                    U_f[0], U_f[1],
                )
            ],
            axis=1,
        )
    ).astype(NP_BF16)
    bs = np.ascontiguousarray(
        np.stack([b.reshape(2, P).T for b in (b_i, b_o, b_u, b_f)], axis=1)
    )

    in_maps = []
    npad = NBLK * 512
    for m in range(NCORES):
        xm = emb[idx[m]]  # [NSLOT, 256]
        arr = np.zeros((256, npad), dtype=np.float32)
        arr[:, :NSLOT] = xm.T
        xtc = np.ascontiguousarray(
            arr.reshape(2, P, npad).transpose(1, 0, 2)
        ).astype(NP_FP8)  # [p, ko, s]
        in_maps.append({"xtb": xtc, "wt8": wt8, "wtb": wtb, "bs": bs})

    kw = {}
    if os.environ.get("KERNEL_TRACE_DIR"):
        kw = {"trace": True, "tmpdir": os.environ["KERNEL_TRACE_DIR"]}
    res = run_bass_kernel_spmd(nc, in_maps, core_ids=list(range(NCORES)), **kw)
    _CACHE["last_res"] = res

    rev = _bitrev(CUT - 3)
    n_cut = 1 << CUT
    h = np.zeros((n_cut, H), dtype=np.float32)
    c = np.zeros((n_cut, H), dtype=np.float32)
    for m in range(NCORES):
        # The device ships the final level's five RAW gate preacts plus the
        # child level's c; finish the level's elementwise math here.
        # [P, ..., 2, N] feature-major -> [N, 2*P] node-major:
        def tonodes(a):
            return a.transpose(2, 1, 0).reshape(a.shape[2], H)

        g = np.asarray(res.results[m]["praw"]).astype(np.float32)
        c14n = tonodes(np.asarray(res.results[m]["hcc"]))  # [2*NOUT, H]
        i_pre, u_pre, f0_pre, f1_pre, o_pre = (tonodes(g[:, k]) for k in range(5))
        ce, co = c14n[:NOUT], c14n[NOUT:]
        c13 = (
            _sigmoid(i_pre + b_i) * np.tanh(u_pre + b_u)
            + _sigmoid(f0_pre + b_f) * ce
            + _sigmoid(f1_pre + b_f) * co
        )
        pos = m * NOUT + rev
        h[pos] = _sigmoid(o_pre + b_o) * np.tanh(c13)
        c[pos] = c13

    prm = (W_i, b_i, U_i, W_o, b_o, U_o, W_u, b_u, U_u, W_f, b_f, U_f)
    for lvl in range(CUT - 1, -1, -1):
        n = 1 << lvl
        start = n - 1
        ch_h = h.reshape(n, 2, H)
        ch_c = c.reshape(n, 2, H)
        h, c = _host_node_batch(emb[start : start + n], ch_h, ch_c, prm)

    return h[0], c[0]
